# revision 9
# baseline (speedup 1.0000x reference)
"""Trainium2 Bass kernel for nn_AttentionHead (B=256, T=200, D_MODEL=2048,
D_KEY=D_VAL=128), data-parallel over batch across 8 NeuronCores.

Per core (32 batches):
  - load q[b] f32 natural ([128,2048] + [80,2048] row tiles, 8-row overlap so
    both tiles stay in-bounds and partition counts work out)
  - cast f32->bf16 on ScalarE
  - PE-transpose q chunks -> q^T [128c, t] bf16 (chunk-major pair tile,
    rhs free dim 400 = 2 batches for the projection matmuls)
  - QKV projections: Q^T,K^T,V^T = W^T.T @ q^T accumulated over 16 c-chunks
  - scores = Q^T.T @ K^T / sqrt(2048) (scale folded into Wq), pad mask folded
    into the scores accumulation as a K=1 matmul adding -30000 on padded keys
  - P = exp(scores) (no max subtraction: logits are provably small), causal
    mask via gpsimd affine_select, denom = rowsum
  - out = (P^T.T @ V) * (1/denom)
All attention matmuls bf16; accumulation f32 in PSUM.
"""

import os
import numpy as np

import concourse.bass as bass
import concourse.bacc as bacc
import concourse.mybir as mybir
from concourse import tile
from concourse import bass_utils

AF = mybir.ActivationFunctionType
ALU = mybir.AluOpType
BF16 = mybir.dt.bfloat16
F32 = mybir.dt.float32
I32 = mybir.dt.int32

N_CORES = 8
B_FULL, T, C = 256, 200, 2048
DK = 128
B_CORE = B_FULL // N_CORES          # 32
NCH = C // 128                      # 16
NPAIR = B_CORE // 2                 # 16
NEG = -30000.0
SCALE = 1.0 / float(np.sqrt(2048.0))

# t-row tiling within a batch: [0:128] and [120:200] (80 rows, overlap 8)
T0, T1, T1OFF = 128, 80, 120


def build_kernel():
    nc = bacc.Bacc("TRN2", target_bir_lowering=False, debug=False,
                   num_devices=N_CORES)
    q_d = nc.dram_tensor("q", [B_CORE * T, C], F32, kind="ExternalInput")
    pm_d = nc.dram_tensor("pm", [1, B_CORE * T], I32, kind="ExternalInput")
    wq_d = nc.dram_tensor("wq", [DK, C], F32, kind="ExternalInput")
    wk_d = nc.dram_tensor("wk", [DK, C], F32, kind="ExternalInput")
    wv_d = nc.dram_tensor("wv", [DK, C], F32, kind="ExternalInput")
    out_d = nc.dram_tensor("out", [B_CORE, T, DK], F32, kind="ExternalOutput")

    with tile.TileContext(nc) as tc:
        with (
            tc.tile_pool(name="const", bufs=1) as constp,
            tc.tile_pool(name="wt", bufs=1) as wtp,
            tc.tile_pool(name="wld", bufs=1) as wldp,
            tc.tile_pool(name="load", bufs=4) as loadp,
            tc.tile_pool(name="cast", bufs=4) as castp,
            tc.tile_pool(name="qt", bufs=3) as qtp,
            tc.tile_pool(name="qkv", bufs=2) as qkvp,
            tc.tile_pool(name="attn", bufs=4) as attnp,
            tc.tile_pool(name="osb", bufs=3) as osbp,
            tc.tile_pool(name="pstage", bufs=3, space="PSUM") as pstagep,
            tc.tile_pool(name="pqkv", bufs=1, space="PSUM") as pqkvp,
            tc.tile_pool(name="pattn", bufs=2, space="PSUM") as pattnp,
        ):
            # ---- constants ----
            ones = constp.tile([128, 128], BF16)
            nc.gpsimd.memset(ones[:], 1.0)
            ident = constp.tile([128, 128], BF16)
            nc.gpsimd.affine_select(
                ident[:], ones[:], pattern=[[-1, 128]], base=0,
                channel_multiplier=1, compare_op=ALU.is_equal, fill=0.0)

            ones1 = constp.tile([1, 256], BF16)
            nc.gpsimd.memset(ones1[:], 1.0)
            zert = constp.tile([128, 200], BF16)
            nc.gpsimd.memset(zert[:], 0.0)
            # causnegT0[p, j] = NEG where j < p (tq < tk) else 0
            causnegT0 = constp.tile([128, 200], BF16)
            nc.gpsimd.affine_select(
                causnegT0[:], zert[:], pattern=[[1, 200]], base=0,
                channel_multiplier=-1, compare_op=ALU.is_ge, fill=NEG)
            # causnegT1[p, j] = NEG where j < p + 8  (tq=120+j < tk=128+p)
            causnegT1 = constp.tile([72, 80], BF16)
            nc.gpsimd.affine_select(
                causnegT1[:], zert[:72, :80], pattern=[[1, 80]], base=-8,
                channel_multiplier=-1, compare_op=ALU.is_ge, fill=NEG)

            # pad mask -> additive -30000 row, all on partition 0
            padneg = constp.tile([1, B_CORE * T], BF16)
            for chk in range(4):
                n = B_CORE * T // 4
                pm_i = wldp.tile([1, n], I32, tag="pmstage")
                nc.sync.dma_start(out=pm_i[:], in_=pm_d.ap()[:, chk * n:(chk + 1) * n])
                nc.vector.tensor_scalar_mul(
                    padneg[:, chk * n:(chk + 1) * n], pm_i[:], NEG)

            # ---- load+cast helper with cache so pair 0 can be hoisted ----
            _lc_cache = {}

            def load_cast(b):
                if b in _lc_cache:
                    return _lc_cache[b]
                row0 = b * T
                ld0 = loadp.tile([T0, C], F32, tag="ld0")
                nc.sync.dma_start(out=ld0[:], in_=q_d.ap()[row0:row0 + T0, :])
                cs0 = castp.tile([T0, C], BF16, tag="cs0")
                nc.vector.tensor_copy(cs0[:], ld0[:])
                ld1 = loadp.tile([T1, C], F32, tag="ld1")
                nc.sync.dma_start(
                    out=ld1[:], in_=q_d.ap()[row0 + T1OFF:row0 + T, :])
                cs1 = castp.tile([T1, C], BF16, tag="cs1")
                nc.gpsimd.tensor_copy(cs1[:], ld1[:])
                _lc_cache[b] = (cs0, cs1)
                return cs0, cs1

            # ---- weights: load f32, cast bf16 (Wq folds softmax scale),
            #      PE-transpose to W^T [128c-part, 16, 128k] ----
            wts = []
            for name, wd, scale in (("wq", wq_d, SCALE), ("wk", wk_d, 1.0),
                                    ("wv", wv_d, 1.0)):
                w_f = wldp.tile([DK, C], F32, tag="wload")
                nc.sync.dma_start(out=w_f[:], in_=wd.ap())
                w_b = wldp.tile([DK, C], BF16, tag="wcast")
                if scale != 1.0:
                    nc.scalar.mul(w_b[:], w_f[:], scale)
                else:
                    nc.scalar.copy(w_b[:], w_f[:])
                wt = wtp.tile([128, NCH, DK], BF16, tag=f"wt_{name}")
                for g in range(4):
                    ps = pattnp.tile([128, 512], BF16, tag="pat")
                    for j in range(4):
                        ch = g * 4 + j
                        nc.tensor.transpose(
                            ps[:, j * 128:(j + 1) * 128],
                            w_b[:, ch * 128:(ch + 1) * 128], ident[:])
                    nc.vector.tensor_copy(wt[:, g * 4:(g + 1) * 4, :], ps[:])
                wts.append(wt)
            wt_q, wt_k, wt_v = wts

            # ---- main loop over batch pairs ----
            for pair in range(NPAIR):
                # --- produce q^T for the pair: [128, 16, 400] bf16 ---
                qt = qtp.tile([128, NCH, 2 * T], BF16, tag="qt")
                for i in range(2):
                    b = pair * 2 + i
                    toff = i * T
                    cs0, cs1 = load_cast(b)

                    # transposes: 8 chunks per PSUM bank, then one drain each
                    for g in range(2):
                        ps = pstagep.tile([128, 1024], BF16, tag="stage")
                        for j in range(8):
                            ch = g * 8 + j
                            nc.tensor.transpose(
                                ps[:, j * 128:(j + 1) * 128],
                                cs0[:, ch * 128:(ch + 1) * 128], ident[:])
                        nc.vector.tensor_copy(
                            qt[:, g * 8:(g + 1) * 8, toff:toff + T0],
                            ps[:].rearrange("p (c t) -> p c t", c=8))
                    for g in range(2):
                        ps = pstagep.tile([128, 1024], BF16, tag="stage")
                        for j in range(8):
                            ch = g * 8 + j
                            nc.tensor.transpose(
                                ps[:, j * 80:(j + 1) * 80],
                                cs1[:, ch * 128:(ch + 1) * 128],
                                ident[:T1, :T1])
                        # drop the 8 overlap cols (t 120:128) on drain
                        nc.scalar.copy(
                            qt[:, g * 8:(g + 1) * 8, toff + T0:toff + T],
                            ps[:, :640].rearrange(
                                "p (c t) -> p c t", c=8)[:, :, 8:])

                # --- QKV projections for the pair ---
                ps_q = pqkvp.tile([128, 2 * T], F32, tag="psq")
                ps_k = pqkvp.tile([128, 2 * T], F32, tag="psk")
                ps_v = pqkvp.tile([128, 2 * T], F32, tag="psv")
                for ch in range(NCH):
                    st, sp = (ch == 0), (ch == NCH - 1)
                    nc.tensor.matmul(ps_q[:], wt_q[:, ch, :], qt[:, ch, :],
                                     start=st, stop=sp)
                    nc.tensor.matmul(ps_k[:], wt_k[:, ch, :], qt[:, ch, :],
                                     start=st, stop=sp)
                    nc.tensor.matmul(ps_v[:], wt_v[:, ch, :], qt[:, ch, :],
                                     start=st, stop=sp)
                qT = qkvp.tile([128, 2 * T], BF16, tag="qT")
                kT = qkvp.tile([128, 2 * T], BF16, tag="kT")
                vT = qkvp.tile([128, 2 * T], BF16, tag="vT")
                nc.vector.tensor_copy(vT[:, 0:T], ps_v[:, 0:T])
                nc.scalar.copy(qT[:, 0:T], ps_q[:, 0:T])
                nc.vector.tensor_copy(kT[:, 0:T], ps_k[:, 0:T])
                nc.vector.tensor_copy(vT[:, T:2 * T], ps_v[:, T:2 * T])
                nc.scalar.copy(qT[:, T:2 * T], ps_q[:, T:2 * T])
                nc.vector.tensor_copy(kT[:, T:2 * T], ps_k[:, T:2 * T])

                # --- attention, both batches interleaved ---
                ps_s_l, pt_l, v_sb_l, ps_o_l = [], [], [], []
                for i in range(2):
                    b = pair * 2 + i
                    o = i * T
                    # scores^T: tile0 [128tk, 200tq] cols 0:200,
                    #           tile1 [72tk, 80tq] cols 200:280
                    ps_s = pattnp.tile([128, 328], F32, tag="pat")
                    nc.tensor.matmul(ps_s[:, 0:200],
                                     padneg[:, b * T:b * T + 128],
                                     ones1[:, :200], start=True, stop=False)
                    nc.tensor.matmul(ps_s[:, 0:200], ident[:], causnegT0[:],
                                     start=False, stop=False)
                    nc.tensor.matmul(ps_s[:, 0:200], kT[:, o:o + 128],
                                     qT[:, o:o + T], start=False, stop=True)
                    nc.tensor.matmul(ps_s[:72, 200:280],
                                     padneg[:, b * T + 128:b * T + T],
                                     ones1[:, :80], start=True, stop=False)
                    nc.tensor.matmul(ps_s[:72, 200:280], ident[:72, :72],
                                     causnegT1[:], start=False, stop=False)
                    nc.tensor.matmul(ps_s[:72, 200:280],
                                     kT[:, o + 128:o + T],
                                     qT[:, o + T1OFF:o + T],
                                     start=False, stop=True)
                    ps_s_l.append(ps_s)
                    # P^T = exp(scores^T)
                    pt = attnp.tile([128, 280], BF16, tag="pt")
                    nc.scalar.activation(pt[:, 0:200], ps_s[:, 0:200], AF.Exp)
                    nc.scalar.activation(pt[:72, 200:280],
                                         ps_s[:72, 200:280], AF.Exp)
                    pt_l.append(pt)
                for i in range(2):
                    o = i * T
                    # V natural + ones col
                    v_sb = attnp.tile([128, 2, 132], BF16, tag="v_sb")
                    psv0 = pstagep.tile([128, 1024], BF16, tag="stage")
                    nc.tensor.transpose(psv0[:, :128], vT[:, o:o + 128],
                                        ident[:])
                    nc.tensor.transpose(psv0[:72, 128:256],
                                        vT[:, o + 128:o + T], ident[:])
                    nc.scalar.copy(v_sb[:, 0, 0:128], psv0[:, :128])
                    nc.scalar.copy(v_sb[:72, 1, 0:128], psv0[:72, 128:256])
                    nc.gpsimd.memset(v_sb[:, 0, 128:129], 1.0)
                    nc.gpsimd.memset(v_sb[:72, 1, 128:129], 1.0)
                    v_sb_l.append(v_sb)
                for i in range(2):
                    pt, v_sb = pt_l[i], v_sb_l[i]
                    ps_o = pattnp.tile([128, 328], F32, tag="pat")
                    nc.tensor.matmul(ps_o[:, 0:132], pt[:, 0:128],
                                     v_sb[:, 0, :], start=True, stop=True)
                    nc.tensor.matmul(ps_o[:80, 132:264], pt[:, T1OFF:T],
                                     v_sb[:, 0, :], start=True, stop=False)
                    nc.tensor.matmul(ps_o[:80, 132:264], pt[:72, 200:280],
                                     v_sb[:72, 1, :], start=False, stop=True)
                    ps_o_l.append(ps_o)
                for i in range(2):
                    b = pair * 2 + i
                    ps_o = ps_o_l[i]
                    rec = attnp.tile([128, 2], F32, tag="rec")
                    nc.vector.reciprocal(rec[:, 0:1], ps_o[:, 128:129])
                    nc.vector.reciprocal(rec[:80, 1:2], ps_o[:80, 260:261])
                    o_sb = osbp.tile([128, 256], F32, tag="o_sb")
                    nc.vector.tensor_scalar_mul(o_sb[:, 0:128], ps_o[:, 0:128],
                                                rec[:, 0:1])
                    nc.vector.tensor_scalar_mul(o_sb[:80, 128:256],
                                                ps_o[:80, 132:260],
                                                rec[:80, 1:2])
                    nc.sync.dma_start(out=out_d.ap()[b, 0:T0, :],
                                      in_=o_sb[:, 0:128])
                    nc.sync.dma_start(out=out_d.ap()[b, T0:T, :],
                                      in_=o_sb[8:80, 128:256])
    nc.compile()
    return nc


_NC_CACHE = None


def kernel(q, pad_mask, Wq, Wk, Wv):
    global _NC_CACHE
    if _NC_CACHE is None:
        _NC_CACHE = build_kernel()
    nc = _NC_CACHE

    q = np.ascontiguousarray(q, dtype=np.float32)
    pad_mask = np.ascontiguousarray(pad_mask, dtype=np.int32)
    Wq = np.ascontiguousarray(Wq, dtype=np.float32)
    Wk = np.ascontiguousarray(Wk, dtype=np.float32)
    Wv = np.ascontiguousarray(Wv, dtype=np.float32)

    in_maps = []
    for c in range(N_CORES):
        sl = slice(c * B_CORE, (c + 1) * B_CORE)
        in_maps.append({
            "q": q[sl].reshape(B_CORE * T, C),
            "pm": pad_mask[sl].reshape(1, B_CORE * T),
            "wq": Wq, "wk": Wk, "wv": Wv,
        })

    trace = bool(int(os.environ.get("KERNEL_TRACE", "0")))
    res = bass_utils.run_bass_kernel_spmd(
        nc, in_maps, core_ids=list(range(N_CORES)), trace=trace)
    if res.exec_time_ns is not None:
        print(f"HW exec time: {res.exec_time_ns} ns")
    out = np.concatenate([r["out"] for r in res.results], axis=0)
    return out


# revision 10
# speedup vs baseline: 1.0026x; 1.0026x over previous
"""Trainium2 Bass kernel for nn_AttentionHead (B=256, T=200, D_MODEL=2048,
D_KEY=D_VAL=128), data-parallel over batch across 8 NeuronCores.

Per core (32 batches):
  - load q[b] f32 natural ([128,2048] + [80,2048] row tiles, 8-row overlap so
    both tiles stay in-bounds and partition counts work out)
  - cast f32->bf16 on ScalarE
  - PE-transpose q chunks -> q^T [128c, t] bf16 (chunk-major pair tile,
    rhs free dim 400 = 2 batches for the projection matmuls)
  - QKV projections: Q^T,K^T,V^T = W^T.T @ q^T accumulated over 16 c-chunks
  - scores = Q^T.T @ K^T / sqrt(2048) (scale folded into Wq), pad mask folded
    into the scores accumulation as a K=1 matmul adding -30000 on padded keys
  - P = exp(scores) (no max subtraction: logits are provably small), causal
    mask via gpsimd affine_select, denom = rowsum
  - out = (P^T.T @ V) * (1/denom)
All attention matmuls bf16; accumulation f32 in PSUM.
"""

import os
import numpy as np

import concourse.bass as bass
import concourse.bacc as bacc
import concourse.mybir as mybir
from concourse import tile
from concourse import bass_utils

AF = mybir.ActivationFunctionType
ALU = mybir.AluOpType
BF16 = mybir.dt.bfloat16
F32 = mybir.dt.float32
I32 = mybir.dt.int32

N_CORES = 8
B_FULL, T, C = 256, 200, 2048
DK = 128
B_CORE = B_FULL // N_CORES          # 32
NCH = C // 128                      # 16
NPAIR = B_CORE // 2                 # 16
NEG = -30000.0
SCALE = 1.0 / float(np.sqrt(2048.0))

# t-row tiling within a batch: [0:128] and [120:200] (80 rows, overlap 8)
T0, T1, T1OFF = 128, 80, 120


def build_kernel():
    nc = bacc.Bacc("TRN2", target_bir_lowering=False, debug=False,
                   num_devices=N_CORES)
    q_d = nc.dram_tensor("q", [B_CORE * T, C], F32, kind="ExternalInput")
    pm_d = nc.dram_tensor("pm", [1, B_CORE * T], I32, kind="ExternalInput")
    wq_d = nc.dram_tensor("wq", [DK, C], F32, kind="ExternalInput")
    wk_d = nc.dram_tensor("wk", [DK, C], F32, kind="ExternalInput")
    wv_d = nc.dram_tensor("wv", [DK, C], F32, kind="ExternalInput")
    out_d = nc.dram_tensor("out", [B_CORE, T, DK], F32, kind="ExternalOutput")

    with tile.TileContext(nc) as tc:
        with (
            tc.tile_pool(name="const", bufs=1) as constp,
            tc.tile_pool(name="wt", bufs=1) as wtp,
            tc.tile_pool(name="wld", bufs=1) as wldp,
            tc.tile_pool(name="load", bufs=4) as loadp,
            tc.tile_pool(name="cast", bufs=4) as castp,
            tc.tile_pool(name="qt", bufs=3) as qtp,
            tc.tile_pool(name="qkv", bufs=2) as qkvp,
            tc.tile_pool(name="attn", bufs=4) as attnp,
            tc.tile_pool(name="osb", bufs=3) as osbp,
            tc.tile_pool(name="pstage", bufs=3, space="PSUM") as pstagep,
            tc.tile_pool(name="pqkv", bufs=1, space="PSUM") as pqkvp,
            tc.tile_pool(name="pattn", bufs=2, space="PSUM") as pattnp,
        ):
            # ---- constants ----
            ones = constp.tile([128, 128], BF16)
            nc.gpsimd.memset(ones[:], 1.0)
            ident = constp.tile([128, 128], BF16)
            nc.gpsimd.affine_select(
                ident[:], ones[:], pattern=[[-1, 128]], base=0,
                channel_multiplier=1, compare_op=ALU.is_equal, fill=0.0)

            ones1 = constp.tile([1, 256], BF16)
            nc.gpsimd.memset(ones1[:], 1.0)
            zert = constp.tile([128, 200], BF16)
            nc.gpsimd.memset(zert[:], 0.0)
            # causnegT0[p, j] = NEG where j < p (tq < tk) else 0
            causnegT0 = constp.tile([128, 200], BF16)
            nc.gpsimd.affine_select(
                causnegT0[:], zert[:], pattern=[[1, 200]], base=0,
                channel_multiplier=-1, compare_op=ALU.is_ge, fill=NEG)
            # causnegT1[p, j] = NEG where j < p + 8  (tq=120+j < tk=128+p)
            causnegT1 = constp.tile([72, 80], BF16)
            nc.gpsimd.affine_select(
                causnegT1[:], zert[:72, :80], pattern=[[1, 80]], base=-8,
                channel_multiplier=-1, compare_op=ALU.is_ge, fill=NEG)

            # pad mask -> additive -30000 row, all on partition 0
            padneg = constp.tile([1, B_CORE * T], BF16)
            for chk in range(4):
                n = B_CORE * T // 4
                pm_i = wldp.tile([1, n], I32, tag="pmstage")
                nc.sync.dma_start(out=pm_i[:], in_=pm_d.ap()[:, chk * n:(chk + 1) * n])
                nc.vector.tensor_scalar_mul(
                    padneg[:, chk * n:(chk + 1) * n], pm_i[:], NEG)

            # ---- load helper with cache so early loads can be prefetched ----
            _ld_cache = {}

            def load_q(b):
                if b in _ld_cache:
                    return _ld_cache[b]
                row0 = b * T
                ld0 = loadp.tile([T0, C], F32, tag="ld0")
                nc.sync.dma_start(out=ld0[:], in_=q_d.ap()[row0:row0 + T0, :])
                ld1 = loadp.tile([T1, C], F32, tag="ld1")
                nc.sync.dma_start(
                    out=ld1[:], in_=q_d.ap()[row0 + T1OFF:row0 + T, :])
                _ld_cache[b] = (ld0, ld1)
                return ld0, ld1

            def load_cast(b):
                ld0, ld1 = load_q(b)
                cs0 = castp.tile([T0, C], BF16, tag="cs0")
                nc.vector.tensor_copy(cs0[:], ld0[:])
                cs1 = castp.tile([T1, C], BF16, tag="cs1")
                nc.gpsimd.tensor_copy(cs1[:], ld1[:])
                return cs0, cs1


            # ---- weights: load f32, cast bf16 (Wq folds softmax scale),
            #      PE-transpose to W^T [128c-part, 16, 128k] ----
            wts = []
            for name, wd, scale in (("wq", wq_d, SCALE), ("wk", wk_d, 1.0),
                                    ("wv", wv_d, 1.0)):
                w_f = wldp.tile([DK, C], F32, tag="wload")
                nc.sync.dma_start(out=w_f[:], in_=wd.ap())
                w_b = wldp.tile([DK, C], BF16, tag="wcast")
                if scale != 1.0:
                    nc.scalar.mul(w_b[:], w_f[:], scale)
                else:
                    nc.scalar.copy(w_b[:], w_f[:])
                wt = wtp.tile([128, NCH, DK], BF16, tag=f"wt_{name}")
                for g in range(4):
                    ps = pattnp.tile([128, 512], BF16, tag="pat")
                    for j in range(4):
                        ch = g * 4 + j
                        nc.tensor.transpose(
                            ps[:, j * 128:(j + 1) * 128],
                            w_b[:, ch * 128:(ch + 1) * 128], ident[:])
                    nc.vector.tensor_copy(wt[:, g * 4:(g + 1) * 4, :], ps[:])
                wts.append(wt)
            wt_q, wt_k, wt_v = wts

            # ---- main loop over batch pairs (attention pipelined 1 behind) ----
            def attention(pair, qT, kT, vT):
                ps_s_l, pt_l, v_sb_l, ps_o_l = [], [], [], []
                for i in range(2):
                    b = pair * 2 + i
                    o = i * T
                    ps_s = pattnp.tile([128, 328], F32, tag="pat")
                    nc.tensor.matmul(ps_s[:, 0:200],
                                     padneg[:, b * T:b * T + 128],
                                     ones1[:, :200], start=True, stop=False)
                    nc.tensor.matmul(ps_s[:, 0:200], ident[:], causnegT0[:],
                                     start=False, stop=False)
                    nc.tensor.matmul(ps_s[:, 0:200], kT[:, o:o + 128],
                                     qT[:, o:o + T], start=False, stop=True)
                    nc.tensor.matmul(ps_s[:72, 200:280],
                                     padneg[:, b * T + 128:b * T + T],
                                     ones1[:, :80], start=True, stop=False)
                    nc.tensor.matmul(ps_s[:72, 200:280], ident[:72, :72],
                                     causnegT1[:], start=False, stop=False)
                    nc.tensor.matmul(ps_s[:72, 200:280],
                                     kT[:, o + 128:o + T],
                                     qT[:, o + T1OFF:o + T],
                                     start=False, stop=True)
                    ps_s_l.append(ps_s)
                    pt = attnp.tile([128, 280], BF16, tag="pt")
                    nc.scalar.activation(pt[:, 0:200], ps_s[:, 0:200], AF.Exp)
                    nc.scalar.activation(pt[:72, 200:280],
                                         ps_s[:72, 200:280], AF.Exp)
                    pt_l.append(pt)
                for i in range(2):
                    o = i * T
                    v_sb = attnp.tile([128, 2, 132], BF16, tag="v_sb")
                    psv0 = pstagep.tile([128, 1024], BF16, tag="stage")
                    nc.tensor.transpose(psv0[:, :128], vT[:, o:o + 128],
                                        ident[:])
                    nc.tensor.transpose(psv0[:72, 128:256],
                                        vT[:, o + 128:o + T], ident[:])
                    nc.scalar.copy(
                        v_sb[:, :, 0:128],
                        psv0[:, :256].rearrange("p (c v) -> p c v", c=2))
                    nc.gpsimd.memset(v_sb[:, 0, 128:129], 1.0)
                    nc.gpsimd.memset(v_sb[:, 1, 128:129], 1.0)
                    v_sb_l.append(v_sb)
                for i in range(2):
                    pt, v_sb = pt_l[i], v_sb_l[i]
                    ps_o = pattnp.tile([128, 328], F32, tag="pat")
                    nc.tensor.matmul(ps_o[:, 0:132], pt[:, 0:128],
                                     v_sb[:, 0, :], start=True, stop=True)
                    nc.tensor.matmul(ps_o[:80, 132:264], pt[:, T1OFF:T],
                                     v_sb[:, 0, :], start=True, stop=False)
                    nc.tensor.matmul(ps_o[:80, 132:264], pt[:72, 200:280],
                                     v_sb[:72, 1, :], start=False, stop=True)
                    ps_o_l.append(ps_o)
                for i in range(2):
                    b = pair * 2 + i
                    ps_o = ps_o_l[i]
                    rec = attnp.tile([128, 2], F32, tag="rec")
                    nc.vector.reciprocal(rec[:, 0:1], ps_o[:, 128:129])
                    nc.vector.reciprocal(rec[:80, 1:2], ps_o[:80, 260:261])
                    o_sb = osbp.tile([128, 256], F32, tag="o_sb")
                    nc.vector.tensor_scalar_mul(o_sb[:, 0:128], ps_o[:, 0:128],
                                                rec[:, 0:1])
                    nc.vector.tensor_scalar_mul(o_sb[:80, 128:256],
                                                ps_o[:80, 132:260],
                                                rec[:80, 1:2])
                    nc.sync.dma_start(out=out_d.ap()[b, 0:T0, :],
                                      in_=o_sb[:, 0:128])
                    nc.sync.dma_start(out=out_d.ap()[b, T0:T, :],
                                      in_=o_sb[8:80, 128:256])

            prev = None
            for pair in range(NPAIR):
                if prev is not None:
                    attention(*prev)
                # --- produce q^T for the pair: [128, 16, 400] bf16 ---
                qt = qtp.tile([128, NCH, 2 * T], BF16, tag="qt")
                for i in range(2):
                    b = pair * 2 + i
                    toff = i * T
                    cs0, cs1 = load_cast(b)

                    for g in range(2):
                        ps = pstagep.tile([128, 1024], BF16, tag="stage")
                        for j in range(8):
                            ch = g * 8 + j
                            nc.tensor.transpose(
                                ps[:, j * 128:(j + 1) * 128],
                                cs0[:, ch * 128:(ch + 1) * 128], ident[:])
                        nc.vector.tensor_copy(
                            qt[:, g * 8:(g + 1) * 8, toff:toff + T0],
                            ps[:].rearrange("p (c t) -> p c t", c=8))
                    for g in range(2):
                        ps = pstagep.tile([128, 1024], BF16, tag="stage")
                        for j in range(8):
                            ch = g * 8 + j
                            nc.tensor.transpose(
                                ps[:, j * 80:(j + 1) * 80],
                                cs1[:, ch * 128:(ch + 1) * 128],
                                ident[:T1, :T1])
                        # drop the 8 overlap cols (t 120:128) on drain
                        nc.scalar.copy(
                            qt[:, g * 8:(g + 1) * 8, toff + T0:toff + T],
                            ps[:, :640].rearrange(
                                "p (c t) -> p c t", c=8)[:, :, 8:])

                # --- QKV projections for the pair ---
                ps_q = pqkvp.tile([128, 2 * T], F32, tag="psq")
                ps_k = pqkvp.tile([128, 2 * T], F32, tag="psk")
                ps_v = pqkvp.tile([128, 2 * T], F32, tag="psv")
                for ch in range(NCH):
                    st, sp = (ch == 0), (ch == NCH - 1)
                    nc.tensor.matmul(ps_q[:], wt_q[:, ch, :], qt[:, ch, :],
                                     start=st, stop=sp)
                    nc.tensor.matmul(ps_k[:], wt_k[:, ch, :], qt[:, ch, :],
                                     start=st, stop=sp)
                    nc.tensor.matmul(ps_v[:], wt_v[:, ch, :], qt[:, ch, :],
                                     start=st, stop=sp)
                qT = qkvp.tile([128, 2 * T], BF16, tag="qT")
                kT = qkvp.tile([128, 2 * T], BF16, tag="kT")
                vT = qkvp.tile([128, 2 * T], BF16, tag="vT")
                nc.vector.tensor_copy(vT[:, 0:T], ps_v[:, 0:T])
                nc.scalar.copy(qT[:, 0:T], ps_q[:, 0:T])
                nc.vector.tensor_copy(kT[:, 0:T], ps_k[:, 0:T])
                nc.vector.tensor_copy(vT[:, T:2 * T], ps_v[:, T:2 * T])
                nc.scalar.copy(qT[:, T:2 * T], ps_q[:, T:2 * T])
                nc.vector.tensor_copy(kT[:, T:2 * T], ps_k[:, T:2 * T])
                prev = (pair, qT, kT, vT)
            attention(*prev)
    nc.compile()
    return nc


_NC_CACHE = None


def kernel(q, pad_mask, Wq, Wk, Wv):
    global _NC_CACHE
    if _NC_CACHE is None:
        _NC_CACHE = build_kernel()
    nc = _NC_CACHE

    q = np.ascontiguousarray(q, dtype=np.float32)
    pad_mask = np.ascontiguousarray(pad_mask, dtype=np.int32)
    Wq = np.ascontiguousarray(Wq, dtype=np.float32)
    Wk = np.ascontiguousarray(Wk, dtype=np.float32)
    Wv = np.ascontiguousarray(Wv, dtype=np.float32)

    in_maps = []
    for c in range(N_CORES):
        sl = slice(c * B_CORE, (c + 1) * B_CORE)
        in_maps.append({
            "q": q[sl].reshape(B_CORE * T, C),
            "pm": pad_mask[sl].reshape(1, B_CORE * T),
            "wq": Wq, "wk": Wk, "wv": Wv,
        })

    trace = bool(int(os.environ.get("KERNEL_TRACE", "0")))
    res = bass_utils.run_bass_kernel_spmd(
        nc, in_maps, core_ids=list(range(N_CORES)), trace=trace)
    if res.exec_time_ns is not None:
        print(f"HW exec time: {res.exec_time_ns} ns")
    out = np.concatenate([r["out"] for r in res.results], axis=0)
    return out


# revision 11
# speedup vs baseline: 1.0148x; 1.0122x over previous
"""Trainium2 Bass kernel for nn_AttentionHead (B=256, T=200, D_MODEL=2048,
D_KEY=D_VAL=128), data-parallel over batch across 8 NeuronCores.

Per core (32 batches):
  - load q[b] f32 natural ([128,2048] + [80,2048] row tiles, 8-row overlap so
    both tiles stay in-bounds and partition counts work out)
  - cast f32->bf16 on ScalarE
  - PE-transpose q chunks -> q^T [128c, t] bf16 (chunk-major pair tile,
    rhs free dim 400 = 2 batches for the projection matmuls)
  - QKV projections: Q^T,K^T,V^T = W^T.T @ q^T accumulated over 16 c-chunks
  - scores = Q^T.T @ K^T / sqrt(2048) (scale folded into Wq), pad mask folded
    into the scores accumulation as a K=1 matmul adding -30000 on padded keys
  - P = exp(scores) (no max subtraction: logits are provably small), causal
    mask via gpsimd affine_select, denom = rowsum
  - out = (P^T.T @ V) * (1/denom)
All attention matmuls bf16; accumulation f32 in PSUM.
"""

import os
import numpy as np

import concourse.bass as bass
import concourse.bacc as bacc
import concourse.mybir as mybir
from concourse import tile
from concourse import bass_utils

AF = mybir.ActivationFunctionType
ALU = mybir.AluOpType
BF16 = mybir.dt.bfloat16
F32 = mybir.dt.float32
I32 = mybir.dt.int32

N_CORES = 8
B_FULL, T, C = 256, 200, 2048
DK = 128
B_CORE = B_FULL // N_CORES          # 32
NCH = C // 128                      # 16
NPAIR = B_CORE // 2                 # 16
NEG = -30000.0
SCALE = 1.0 / float(np.sqrt(2048.0))

# t-row tiling within a batch: [0:128] and [120:200] (80 rows, overlap 8)
T0, T1, T1OFF = 128, 80, 120


def build_kernel():
    nc = bacc.Bacc("TRN2", target_bir_lowering=False, debug=False,
                   num_devices=N_CORES)
    q_d = nc.dram_tensor("q", [B_CORE * T, C], F32, kind="ExternalInput")
    pm_d = nc.dram_tensor("pm", [1, B_CORE * T], I32, kind="ExternalInput")
    wq_d = nc.dram_tensor("wq", [DK, C], F32, kind="ExternalInput")
    wk_d = nc.dram_tensor("wk", [DK, C], F32, kind="ExternalInput")
    wv_d = nc.dram_tensor("wv", [DK, C], F32, kind="ExternalInput")
    out_d = nc.dram_tensor("out", [B_CORE, T, DK], F32, kind="ExternalOutput")

    with tile.TileContext(nc) as tc:
        with (
            tc.tile_pool(name="const", bufs=1) as constp,
            tc.tile_pool(name="wt", bufs=1) as wtp,
            tc.tile_pool(name="wld", bufs=1) as wldp,
            tc.tile_pool(name="load", bufs=4) as loadp,
            tc.tile_pool(name="cast", bufs=4) as castp,
            tc.tile_pool(name="qt", bufs=3) as qtp,
            tc.tile_pool(name="qkv", bufs=2) as qkvp,
            tc.tile_pool(name="attn", bufs=4) as attnp,
            tc.tile_pool(name="osb", bufs=3) as osbp,
            tc.tile_pool(name="pstage", bufs=3, space="PSUM") as pstagep,
            tc.tile_pool(name="pqkv", bufs=1, space="PSUM") as pqkvp,
            tc.tile_pool(name="pattn", bufs=2, space="PSUM") as pattnp,
        ):
            # ---- constants ----
            ones = constp.tile([128, 128], BF16)
            nc.gpsimd.memset(ones[:], 1.0)
            ident = constp.tile([128, 128], BF16)
            nc.gpsimd.affine_select(
                ident[:], ones[:], pattern=[[-1, 128]], base=0,
                channel_multiplier=1, compare_op=ALU.is_equal, fill=0.0)

            ones1 = constp.tile([1, 256], BF16)
            nc.gpsimd.memset(ones1[:], 1.0)
            zert = constp.tile([128, 200], BF16)
            nc.gpsimd.memset(zert[:], 0.0)
            # causnegT0[p, j] = NEG where j < p (tq < tk) else 0
            causnegT0 = constp.tile([128, 200], BF16)
            nc.gpsimd.affine_select(
                causnegT0[:], zert[:], pattern=[[1, 200]], base=0,
                channel_multiplier=-1, compare_op=ALU.is_ge, fill=NEG)
            # causnegT1[p, j] = NEG where j < p + 8  (tq=120+j < tk=128+p)
            causnegT1 = constp.tile([72, 80], BF16)
            nc.gpsimd.affine_select(
                causnegT1[:], zert[:72, :80], pattern=[[1, 80]], base=-8,
                channel_multiplier=-1, compare_op=ALU.is_ge, fill=NEG)

            # pad mask -> additive -30000 row, all on partition 0
            padneg = constp.tile([1, B_CORE * T], BF16)
            for chk in range(4):
                n = B_CORE * T // 4
                pm_i = wldp.tile([1, n], I32, tag="pmstage")
                nc.sync.dma_start(out=pm_i[:], in_=pm_d.ap()[:, chk * n:(chk + 1) * n])
                nc.vector.tensor_scalar_mul(
                    padneg[:, chk * n:(chk + 1) * n], pm_i[:], NEG)

            # ---- load helper with cache so early loads can be prefetched ----
            _ld_cache = {}

            def load_q(b):
                if b in _ld_cache:
                    return _ld_cache[b]
                row0 = b * T
                ld0 = loadp.tile([T0, C], F32, tag="ld0")
                nc.sync.dma_start(out=ld0[:], in_=q_d.ap()[row0:row0 + T0, :])
                ld1 = loadp.tile([T1, C], F32, tag="ld1")
                nc.sync.dma_start(
                    out=ld1[:], in_=q_d.ap()[row0 + T1OFF:row0 + T, :])
                _ld_cache[b] = (ld0, ld1)
                return ld0, ld1

            def load_cast(b):
                ld0, ld1 = load_q(b)
                cs0 = castp.tile([T0, C], BF16, tag="cs0")
                nc.vector.tensor_copy(cs0[:], ld0[:])
                cs1 = castp.tile([T1, C], BF16, tag="cs1")
                nc.gpsimd.tensor_copy(cs1[:], ld1[:])
                return cs0, cs1


            # ---- weights: load f32, cast bf16 (Wq folds softmax scale),
            #      PE-transpose to W^T [128c-part, 16, 128k] ----
            wts = []
            for name, wd, scale in (("wq", wq_d, SCALE), ("wk", wk_d, 1.0),
                                    ("wv", wv_d, 1.0)):
                w_f = wldp.tile([DK, C], F32, tag="wload")
                nc.sync.dma_start(out=w_f[:], in_=wd.ap())
                w_b = wldp.tile([DK, C], BF16, tag="wcast")
                if scale != 1.0:
                    nc.scalar.mul(w_b[:], w_f[:], scale)
                else:
                    nc.scalar.copy(w_b[:], w_f[:])
                wt = wtp.tile([128, NCH, DK], BF16, tag=f"wt_{name}")
                for g in range(4):
                    ps = pattnp.tile([128, 512], BF16, tag="pat")
                    for j in range(4):
                        ch = g * 4 + j
                        nc.tensor.transpose(
                            ps[:, j * 128:(j + 1) * 128],
                            w_b[:, ch * 128:(ch + 1) * 128], ident[:])
                    nc.vector.tensor_copy(wt[:, g * 4:(g + 1) * 4, :], ps[:])
                wts.append(wt)
            wt_q, wt_k, wt_v = wts

            # ---- main loop over batch pairs (attention pipelined 1 behind) ----
            def attention(pair, qT, kT, vT):
                ps_s_l, pt_l, v_sb_l, ps_o_l = [], [], [], []
                for i in range(2):
                    b = pair * 2 + i
                    o = i * T
                    ps_s = pattnp.tile([128, 328], F32, tag="pat")
                    nc.tensor.matmul(ps_s[:, 0:200],
                                     padneg[:, b * T:b * T + 128],
                                     ones1[:, :200], start=True, stop=False)
                    nc.tensor.matmul(ps_s[:, 0:200], ident[:], causnegT0[:],
                                     start=False, stop=False)
                    nc.tensor.matmul(ps_s[:, 0:200], kT[:, o:o + 128],
                                     qT[:, o:o + T], start=False, stop=True)
                    nc.tensor.matmul(ps_s[:72, 200:280],
                                     padneg[:, b * T + 128:b * T + T],
                                     ones1[:, :80], start=True, stop=False)
                    nc.tensor.matmul(ps_s[:72, 200:280], ident[:72, :72],
                                     causnegT1[:], start=False, stop=False)
                    nc.tensor.matmul(ps_s[:72, 200:280],
                                     kT[:, o + 128:o + T],
                                     qT[:, o + T1OFF:o + T],
                                     start=False, stop=True)
                    ps_s_l.append(ps_s)
                    pt = attnp.tile([128, 280], BF16, tag="pt")
                    nc.scalar.activation(pt[:, 0:200], ps_s[:, 0:200], AF.Exp)
                    nc.scalar.activation(pt[:72, 200:280],
                                         ps_s[:72, 200:280], AF.Exp)
                    pt_l.append(pt)
                for i in range(2):
                    o = i * T
                    v_sb = attnp.tile([128, 2, 132], BF16, tag="v_sb")
                    psv0 = pstagep.tile([128, 1024], BF16, tag="stage")
                    nc.tensor.transpose(psv0[:, :128], vT[:, o:o + 128],
                                        ident[:])
                    nc.tensor.transpose(psv0[:72, 128:256],
                                        vT[:, o + 128:o + T], ident[:])
                    nc.scalar.copy(
                        v_sb[:, :, 0:128],
                        psv0[:, :256].rearrange("p (c v) -> p c v", c=2))
                    nc.gpsimd.memset(v_sb[:, 0, 128:129], 1.0)
                    nc.gpsimd.memset(v_sb[:, 1, 128:129], 1.0)
                    v_sb_l.append(v_sb)
                for i in range(2):
                    pt, v_sb = pt_l[i], v_sb_l[i]
                    ps_o = pattnp.tile([128, 328], F32, tag="pat")
                    nc.tensor.matmul(ps_o[:, 0:132], pt[:, 0:128],
                                     v_sb[:, 0, :], start=True, stop=True)
                    nc.tensor.matmul(ps_o[:80, 132:264], pt[:, T1OFF:T],
                                     v_sb[:, 0, :], start=True, stop=False)
                    nc.tensor.matmul(ps_o[:80, 132:264], pt[:72, 200:280],
                                     v_sb[:72, 1, :], start=False, stop=True)
                    ps_o_l.append(ps_o)
                for i in range(2):
                    b = pair * 2 + i
                    ps_o = ps_o_l[i]
                    rec = attnp.tile([128, 2], F32, tag="rec")
                    nc.vector.reciprocal(rec[:, 0:1], ps_o[:, 128:129])
                    nc.vector.reciprocal(rec[:80, 1:2], ps_o[:80, 260:261])
                    o_sb = osbp.tile([128, 256], F32, tag="o_sb")
                    nc.vector.tensor_scalar_mul(o_sb[:, 0:128], ps_o[:, 0:128],
                                                rec[:, 0:1])
                    nc.vector.tensor_scalar_mul(o_sb[:80, 128:256],
                                                ps_o[:80, 132:260],
                                                rec[:80, 1:2])
                    nc.sync.dma_start(out=out_d.ap()[b, 0:T0, :],
                                      in_=o_sb[:, 0:128])
                    nc.sync.dma_start(out=out_d.ap()[b, T0:T, :],
                                      in_=o_sb[8:80, 128:256])

            prev = None
            for pair in range(NPAIR):
                if prev is not None:
                    attention(*prev)
                # --- produce q^T for the pair: [128, 16, 400] bf16 ---
                qt = qtp.tile([128, NCH, 2 * T], BF16, tag="qt")
                cs = [load_cast(pair * 2), load_cast(pair * 2 + 1)]
                for g in range(2):
                    for i in range(2):
                        toff = i * T
                        cs0, cs1 = cs[i]
                        ps = pstagep.tile([128, 1024], BF16, tag="stage")
                        for j in range(8):
                            ch = g * 8 + j
                            nc.tensor.transpose(
                                ps[:, j * 128:(j + 1) * 128],
                                cs0[:, ch * 128:(ch + 1) * 128], ident[:])
                        nc.vector.tensor_copy(
                            qt[:, g * 8:(g + 1) * 8, toff:toff + T0],
                            ps[:].rearrange("p (c t) -> p c t", c=8))
                        ps = pstagep.tile([128, 1024], BF16, tag="stage")
                        for j in range(8):
                            ch = g * 8 + j
                            nc.tensor.transpose(
                                ps[:, j * 80:(j + 1) * 80],
                                cs1[:, ch * 128:(ch + 1) * 128],
                                ident[:T1, :T1])
                        # drop the 8 overlap cols (t 120:128) on drain
                        nc.scalar.copy(
                            qt[:, g * 8:(g + 1) * 8, toff + T0:toff + T],
                            ps[:, :640].rearrange(
                                "p (c t) -> p c t", c=8)[:, :, 8:])

                # --- QKV projections for the pair ---
                ps_q = pqkvp.tile([128, 2 * T], F32, tag="psq")
                ps_k = pqkvp.tile([128, 2 * T], F32, tag="psk")
                ps_v = pqkvp.tile([128, 2 * T], F32, tag="psv")
                for ch in range(NCH):
                    st, sp = (ch == 0), (ch == NCH - 1)
                    nc.tensor.matmul(ps_q[:], wt_q[:, ch, :], qt[:, ch, :],
                                     start=st, stop=sp)
                    nc.tensor.matmul(ps_k[:], wt_k[:, ch, :], qt[:, ch, :],
                                     start=st, stop=sp)
                    nc.tensor.matmul(ps_v[:], wt_v[:, ch, :], qt[:, ch, :],
                                     start=st, stop=sp)
                qT = qkvp.tile([128, 2 * T], BF16, tag="qT")
                kT = qkvp.tile([128, 2 * T], BF16, tag="kT")
                vT = qkvp.tile([128, 2 * T], BF16, tag="vT")
                nc.vector.tensor_copy(vT[:, 0:T], ps_v[:, 0:T])
                nc.scalar.copy(qT[:, 0:T], ps_q[:, 0:T])
                nc.vector.tensor_copy(kT[:, 0:T], ps_k[:, 0:T])
                nc.vector.tensor_copy(vT[:, T:2 * T], ps_v[:, T:2 * T])
                nc.scalar.copy(qT[:, T:2 * T], ps_q[:, T:2 * T])
                nc.vector.tensor_copy(kT[:, T:2 * T], ps_k[:, T:2 * T])
                prev = (pair, qT, kT, vT)
            attention(*prev)
    nc.compile()
    return nc


_NC_CACHE = None


def kernel(q, pad_mask, Wq, Wk, Wv):
    global _NC_CACHE
    if _NC_CACHE is None:
        _NC_CACHE = build_kernel()
    nc = _NC_CACHE

    q = np.ascontiguousarray(q, dtype=np.float32)
    pad_mask = np.ascontiguousarray(pad_mask, dtype=np.int32)
    Wq = np.ascontiguousarray(Wq, dtype=np.float32)
    Wk = np.ascontiguousarray(Wk, dtype=np.float32)
    Wv = np.ascontiguousarray(Wv, dtype=np.float32)

    in_maps = []
    for c in range(N_CORES):
        sl = slice(c * B_CORE, (c + 1) * B_CORE)
        in_maps.append({
            "q": q[sl].reshape(B_CORE * T, C),
            "pm": pad_mask[sl].reshape(1, B_CORE * T),
            "wq": Wq, "wk": Wk, "wv": Wv,
        })

    trace = bool(int(os.environ.get("KERNEL_TRACE", "0")))
    res = bass_utils.run_bass_kernel_spmd(
        nc, in_maps, core_ids=list(range(N_CORES)), trace=trace)
    if res.exec_time_ns is not None:
        print(f"HW exec time: {res.exec_time_ns} ns")
    out = np.concatenate([r["out"] for r in res.results], axis=0)
    return out
